# revision 34
# baseline (speedup 1.0000x reference)
"""Trainium2 Bass kernel for nn_ConvEmbedder.

out[b, p, e] = (patch(b, p) . conv_w + conv_b) * lin_w[e] + lin_b[e]

data [64, 512, 512] f32, non-overlapping 16x16 patches (1024 per image),
E = 768.  Pure data-parallel over the batch: 8 images per NeuronCore.

Per-core pipeline, one unit = one 128-row group of one image (256 patches,
32 units per core), so every stage streams at fine granularity:
  1. DMA image -> SBUF [128, 4, 512] (SWDGE ring; row-group g = rows 128g..)
  2. DVE: d[:, g, :] *= wexp (in place)   (wexp[p, w] = conv_w[p%16, w%16])
  3. DVE reduce over c-groups: t3[128, 32],  t3[p, j] = sum_c d[p, g, 16j+c]
  4. PE: one tiny matmul with block-diagonal ones lhsT [128, 8] -> PSUM pv[8, 32]
       pv[ii, j] = sum_r t3[16*ii+r, j] = conv_val(patch(8g+ii, j)) - conv_b
     (fp32 matmuls cost 4 cyc/row, so keep the PE streaming dim small: the
      c-reduce runs on DVE *before* the PE row-reduce.)
  5. ScalarE copies pv -> SBUF (f32r round); sync-ring DMAs flatten it into
     lhsT[1, 0:256] (patch-major) and fill lhsT[0, :] = 1.0 from a ones input
  6. PE per 128-patch block: [1; v].T @ [lin_b + conv_b*lin_w; lin_w] -> PSUM
     [128, 768] as float32r (1 cyc/row vs 4 for fp32)
  7. ScalarE copies PSUM -> SBUF
  8. one contiguous 768 KB DMA store per unit (scalar HWDGE ring)
"""

import os

import numpy as np

import concourse.bacc as bacc
import concourse.tile as tile
from concourse import mybir
from concourse.bass_utils import run_bass_kernel_spmd

# fp32 matmuls stream at 4 cyc/row on the PE (2 half-speed passes); float32r
# streams at 1 cyc/row for N>=256.  The expansion matmul is a rank-2 product
# (v x lin_w + 1 x lin_b) so reduced product precision only touches ~1 ulp-ish
# terms; conv stays full fp32.  Toggle for experiments: EXP_F32R=0.
EXP_F32R = os.environ.get("EXP_F32R", "1") == "1"

KS = 16          # conv kernel == patch size
E = 768          # embed dim
NCORES = 8
B = 64
H = 512
W = 512
BPC = B // NCORES          # images per core
NH = H // KS               # 32 patch rows (and patch cols) per image
NPATCH = NH * NH           # 1024 patches per image
NG = H // 128              # 4 row-groups per image
NBLK = NPATCH // 128       # 8 expansion blocks of 128 patches
IBLK = 128 // KS           # 8 patch-rows per row-group
UNIT = IBLK * NH           # 256 patches per unit (one row-group)
DT = mybir.dt.float32
SPLIT = 0                  # DVE copies cols [0:SPLIT), ScalarE [SPLIT:768)

_NC = None
_LAST_RESULTS = None       # BassKernelResults of the last run (for test harness)


def _build_nc(reps=None):
    # reps: bench-only — wrap the whole per-image pipeline in a HW For_i loop
    # so one NEFF launch amortizes the (huge) axon dispatch overhead.
    mm_dt = mybir.dt.float32r if EXP_F32R else DT
    nc = bacc.Bacc("TRN2", target_bir_lowering=False, debug=False)
    data_t = nc.dram_tensor("data", [BPC, H, W], DT, kind="ExternalInput")
    wexp_t = nc.dram_tensor("wexp", [128, W], DT, kind="ExternalInput")
    bd_t = nc.dram_tensor("bd", [128, IBLK], DT, kind="ExternalInput")
    lwb_t = nc.dram_tensor("lwb", [2, E], mm_dt, kind="ExternalInput")
    ones_t = nc.dram_tensor("ones", [1, UNIT], mm_dt, kind="ExternalInput")
    out_t = nc.dram_tensor("out", [BPC, NPATCH, E], DT, kind="ExternalOutput")

    with tile.TileContext(nc) as tc:
        with (
            tc.tile_pool(name="singles", bufs=1) as singles,
            tc.tile_pool(name="dpool", bufs=3) as dpool,
            tc.tile_pool(name="t3pool", bufs=4) as t3pool,
            tc.tile_pool(name="vpool", bufs=4) as vpool,
            tc.tile_pool(name="lpool", bufs=4) as lpool,
            tc.tile_pool(name="opool", bufs=6) as opool,
            tc.tile_pool(name="pvp", bufs=2, space="PSUM") as pvp,
            tc.tile_pool(name="pop", bufs=3, space="PSUM") as pop,
        ):
            wexp = singles.tile([128, W], DT)
            nc.sync.dma_start(out=wexp[:], in_=wexp_t.ap())
            bd = singles.tile([128, IBLK], DT)
            nc.sync.dma_start(out=bd[:], in_=bd_t.ap())
            lwb = singles.tile([2, E], mm_dt)
            nc.sync.dma_start(out=lwb[:], in_=lwb_t.ap())

            def emit_images():
              for img in range(BPC):
                # 1. load image: d[p, g, w] = data[img, 128*g + p, w]
                #    (SWDGE ring so bulk loads never queue ahead of the
                #     latency-critical sync-ring gathers)
                d = dpool.tile([128, NG, W], DT)
                nc.gpsimd.dma_start(
                    out=d[:],
                    in_=data_t.ap()[img].rearrange("(g p) w -> p g w", p=128),
                )
                for g in range(NG):
                    # 2. elementwise conv-weight multiply (in place)
                    nc.vector.tensor_mul(d[:, g, :], d[:, g, :], wexp[:])
                    # 3. reduce the 16 cols of each patch on DVE (keeps the
                    #    PE streaming dim small: fp32 matmul is 4 cyc/row)
                    t3 = t3pool.tile([128, NH], DT)
                    nc.vector.tensor_reduce(
                        out=t3[:],
                        in_=d[:, g, :].rearrange("p (j c) -> p j c", c=KS),
                        axis=mybir.AxisListType.X,
                        op=mybir.AluOpType.add,
                    )
                    # 4. reduce the 16 rows of each patch-row: block-diag ones
                    pv = pvp.tile([IBLK, NH], DT)
                    nc.tensor.matmul(pv[:], bd[:], t3[:], start=True, stop=True)
                    v8 = vpool.tile([IBLK, NH], mm_dt)
                    nc.scalar.copy(v8[:], pv[:])
                    # 5. row 0 = ones, row 1 = v flattened patch-major
                    lhsT = lpool.tile([2, UNIT], mm_dt)
                    nc.sync.dma_start(out=lhsT[0:1, :], in_=ones_t.ap())
                    nc.sync.dma_start(
                        out=lhsT[1:2, :].rearrange("o (i j) -> o i j", j=NH),
                        in_=v8[:],
                    )
                    # 6-8. expansion: out[p, e] = v[p]*lin_w[e] + lin_b_eff[e]
                    ot = opool.tile([128, 2, E], DT)
                    for sub in range(2):
                        lhsT_blk = lhsT[:, 128 * sub:128 * (sub + 1)]
                        po = pop.tile([128, E], DT)
                        nc.tensor.matmul(
                            po[:, 0:512], lhsT_blk, lwb[:, 0:512],
                            start=True, stop=True,
                        )
                        nc.tensor.matmul(
                            po[:, 512:E], lhsT_blk, lwb[:, 512:E],
                            start=True, stop=True,
                        )
                        if SPLIT > 0:
                            nc.vector.tensor_copy(ot[:, sub, 0:SPLIT], po[:, 0:SPLIT])
                        nc.scalar.copy(ot[:, sub, SPLIT:E], po[:, SPLIT:E])
                    nc.scalar.dma_start(
                        out=out_t.ap()[img, UNIT * g:UNIT * (g + 1), :]
                        .rearrange("(blk p) e -> p blk e", p=128),
                        in_=ot[:],
                    )

            if reps is None:
                emit_images()
            else:
                with tc.For_i(0, reps, 1):
                    emit_images()
    nc.compile()
    return nc


def _get_nc():
    global _NC
    if _NC is None:
        _NC = _build_nc()
    return _NC


def _prepare_in_maps(data, conv_w, conv_b, lin_w, lin_b):
    data = np.ascontiguousarray(np.asarray(data, dtype=np.float32))
    conv_w = np.asarray(conv_w, dtype=np.float32).reshape(KS, KS)
    conv_b = np.float32(np.asarray(conv_b, dtype=np.float32))
    lin_w = np.asarray(lin_w, dtype=np.float32).reshape(E)
    lin_b = np.asarray(lin_b, dtype=np.float32).reshape(E)

    # wexp[p, w] = conv_w[p % 16, w % 16]
    wexp = np.ascontiguousarray(np.tile(conv_w, (128 // KS, W // KS)))
    # bd[row, m] = 1 iff m == row//16  (block-diagonal ones, group-invariant)
    bd = np.zeros((128, IBLK), dtype=np.float32)
    rows = np.arange(128)
    bd[rows, rows // KS] = 1.0
    # fold conv_b: v*lin_w + (conv_b*lin_w + lin_b)
    lin_b_eff = (
        np.float64(conv_b) * lin_w.astype(np.float64) + lin_b.astype(np.float64)
    ).astype(np.float32)
    lwb = np.ascontiguousarray(np.stack([lin_b_eff, lin_w], axis=0))
    ones = np.ones((1, UNIT), dtype=np.float32)

    return [
        {
            "data": np.ascontiguousarray(data[i * BPC:(i + 1) * BPC]),
            "wexp": wexp,
            "bd": bd,
            "lwb": lwb,
            "ones": ones,
        }
        for i in range(NCORES)
    ]


def kernel(data, conv_w, conv_b, lin_w, lin_b):
    global _LAST_RESULTS
    in_maps = _prepare_in_maps(data, conv_w, conv_b, lin_w, lin_b)
    nc = _get_nc()
    res = run_bass_kernel_spmd(nc, in_maps, core_ids=list(range(NCORES)))
    _LAST_RESULTS = res
    return np.concatenate([r["out"] for r in res.results], axis=0)
